# revision 38
# baseline (speedup 1.0000x reference)
"""Trainium2 Bass kernel for HandDecoder-style GNN message passing.

Math (per batch element b, N=128 nodes):
  f = relu(MLP3([feature, coords]))                          # [N, C1=32]
  t1[i,j,h] = relu(a[j,h] + kb1[h] - a[i,h]),  a = coords @ kw1   # [N,N,8]
  t2[i,j,k] = relu(sum_h t1[i,j,h] kw2[h,k] + kb2[k])             # [N,N,16]
  g[j,k,d]  = sum_c f[j,c] kw3[k, c*16+d]                          # [N,16,16]
  out[i,d]  = relu(sum_{j,k} t2[i,j,k] g[j,k,d] + sum_c F[c] kb3[c*16+d])
  (F[c] = sum_j f[j,c])

v7 (v6 40us, baseline 133us):
  - out PSUM pre-zeroed by a DVE memset; ALL final matmuls are start=False
    accumulates, hence order-free -> interleaved right after each t2 drain.
    (start=True resets more than its own column range when multiple chains
    share a PSUM bank -- measured 5.3e-2 error -- so no start flags at all.)
  - weight blob split: small decode/t1 weights land first, W2/kw3p second;
    t1/t2 start ~2us earlier
  - flat pool layout sized to exactly 8 PSUM banks so no pool-handoff stalls
  - t2: 16 bf16 matmuls into [128,1024] 2-bank PSUM tiles, 8 Act drains
  - g permute: scratch [j,k,b,d] (j-stride=16*k-stride) -> 1 clean write +
    1 affine gather read
Data-parallel over batch: 4 batch elements per core, 8 cores.
"""

import sys
import numpy as np

for _p in ("/opt/trn_rl_repo",):
    if _p not in sys.path:
        sys.path.insert(0, _p)

import concourse.bass as bass
import concourse.bacc as bacc
import concourse.mybir as mybir
import concourse.tile as tile
from concourse.bass_utils import run_bass_kernel_spmd

import ml_dtypes

B, N = 32, 128
C0, C1, C2 = 64, 32, 16
NCORES = 8
BPC = B // NCORES          # batches per core = 4
F32 = mybir.dt.float32
BF16 = mybir.dt.bfloat16
RELU = mybir.ActivationFunctionType.Relu
COPY = mybir.ActivationFunctionType.Copy
ADD = mybir.AluOpType.add
MAX = mybir.AluOpType.max
BNP = ml_dtypes.bfloat16

_CACHED_NC = None


def build_nc(stage=5):
    import os
    stage = int(os.environ.get("KSTAGE", stage))
    nc = bacc.Bacc("TRN2", target_bir_lowering=False, debug=False,
                   num_devices=NCORES)

    # -------- DRAM I/O --------
    # wbs (small, lands first) [67, 352] bf16:
    #   [0:32]   dw1 rows 0:67
    #   [32:48]  dw2 rows 0:32
    #   [48:80]  dw3 rows 0:16
    #   [80:208] kw1nrep rows 0:4: [-kw1 tiled 16x along (jl,h); row3=0]
    #   [208:336] kw1blk rows 0:64: kw1blk[jl*4+x, jl2*8+h] = kw1b[x,h]*(jl==jl2)
    #   [336:352] kb3r rows 0:32: kb3r[c, d] = kb3[c*16+d]
    wbs = nc.dram_tensor("wbs", [67, 352], BF16, kind="ExternalInput").ap()
    # wbb (big, needed from t2/g on) [128, 512] bf16:
    #   [0:256]  W2: W2[jl*8+h, half*128+jl8*16+k] = kw2[h,k] if jl==half*8+jl8
    #   [256:512] kw3p rows 0:32: kw3p[c, k*16+d] = kw3[k, c*16+d]
    wbb = nc.dram_tensor("wbb", [128, 512], BF16, kind="ExternalInput").ap()
    # wb_f32 [128, 4]: col0 db1(0:32), col1 db2(0:16), col2 db3(0:32), col3 kb2t
    wb_f32 = nc.dram_tensor("wb_f32", [128, 4], F32, kind="ExternalInput").ap()
    xT = nc.dram_tensor("xT", [67, BPC * N], BF16, kind="ExternalInput").ap()
    c4T = nc.dram_tensor("c4T", [4, BPC * N], BF16, kind="ExternalInput").ap()
    c4T2 = nc.dram_tensor("c4T2", [64, BPC * 8], BF16, kind="ExternalInput").ap()
    out_d = nc.dram_tensor("out", [C2, BPC * N], BF16, kind="ExternalOutput").ap()
    # g scratch: [j, k, b, d]-major (j-stride 1024 = 16 * k-stride 64)
    g_dram = nc.dram_tensor("gscr", [128 * BPC * 256], BF16).ap()

    with tile.TileContext(nc) as tc:
        with (
            tc.tile_pool(name="const", bufs=1) as cpool,
            tc.tile_pool(name="work", bufs=1) as wpool,
            tc.tile_pool(name="ps_misc", bufs=1,
                         space=bass.MemorySpace.PSUM) as pmisc,
            tc.tile_pool(name="ps_t2", bufs=2,
                         space=bass.MemorySpace.PSUM) as pt2,
            tc.tile_pool(name="ps_g", bufs=1,
                         space=bass.MemorySpace.PSUM) as pg,
            tc.tile_pool(name="ps_out", bufs=1,
                         space=bass.MemorySpace.PSUM) as pout,
        ):
            wbs_s = cpool.tile([67, 352], BF16, tag="wbs")
            wbb_s = cpool.tile([128, 512], BF16, tag="wbb")
            wf = cpool.tile([128, 4], F32, tag="wf")
            xT_s = cpool.tile([67, BPC * N], BF16, tag="xT")
            c4T_s = cpool.tile([4, BPC * N], BF16, tag="c4T")
            c4T2_s = cpool.tile([64, BPC * 8], BF16, tag="c4T2")
            nc.sync.dma_start(wbs_s[:], wbs)
            nc.sync.dma_start(c4T_s[:], c4T)
            nc.sync.dma_start(c4T2_s[:], c4T2)
            nc.sync.dma_start(wbb_s[:], wbb)
            nc.scalar.dma_start(wf[:], wb_f32)
            nc.scalar.dma_start(xT_s[:], xT)

            dw1 = wbs_s[0:67, 0:32]
            dw2 = wbs_s[0:32, 32:48]
            dw3 = wbs_s[0:16, 48:80]
            kw1nrep = wbs_s[0:4, 80:208]
            kw1blk = wbs_s[0:64, 208:336]
            kb3r = wbs_s[0:32, 336:352]
            W2 = wbb_s[0:128, 0:256]
            kw3p = wbb_s[0:32, 256:512]
            db1 = wf[0:32, 0:1]
            db2 = wf[0:16, 1:2]
            db3 = wf[0:32, 2:3]
            kb2t = wf[0:128, 3:4]

            rep_sb = wpool.tile([128, BPC * N], F32, tag="repsb")
            a2bT_sb = wpool.tile([128, BPC * 8], F32, tag="a2bT")
            h1 = wpool.tile([32, BPC * N], BF16, tag="h1")
            h2 = wpool.tile([16, BPC * N], BF16, tag="h2")
            fT = wpool.tile([32, BPC * N], BF16, tag="fT")
            g_all = wpool.tile([128, BPC * 256], BF16, tag="gall")
            t2ball = wpool.tile([128, BPC * 2048], BF16, tag="t2ball")
            t1t = [wpool.tile([128, BPC * N], BF16, name=f"t1_{jc}",
                              tag=f"t1_{jc}") for jc in range(8)]

            # out accumulator: pre-zeroed; all final matmuls accumulate.
            ot_ps = pout.tile([16, BPC * N], F32, tag="ot")
            nc.vector.memset(ot_ps[:], 0.0)

            # ---- decode d1 first (longest dependency chain to g_all) ----
            d1_ps = pmisc.tile([32, BPC * N], F32, tag="m")
            nc.tensor.matmul(d1_ps[:], dw1, xT_s[:])
            nc.scalar.activation(h1[:], d1_ps[:], RELU, bias=db1)

            # ---- a-stage: rep & a2bT (gate all t1) ----
            rep_ps = pmisc.tile([128, BPC * N], F32, tag="m")
            nc.tensor.matmul(rep_ps[:], kw1nrep, c4T_s[:])
            nc.vector.tensor_copy(rep_sb[:], rep_ps[:])
            a2bT_ps = pmisc.tile([128, BPC * 8], F32, tag="m")
            nc.tensor.matmul(a2bT_ps[:], kw1blk, c4T2_s[:])
            nc.vector.tensor_copy(a2bT_sb[:], a2bT_ps[:])

            # ---- t1: 32 DVE ops; g casts are queued after jc=4 so the
            # g bounce isn't stuck behind the whole t1 stream ----
            def emit_t1(jcs):
                for jc in jcs:
                    for b in range(BPC):
                        nc.vector.tensor_scalar(
                            t1t[jc][:, b * N:(b + 1) * N],
                            rep_sb[:, b * N:(b + 1) * N],
                            a2bT_sb[:, b * 8 + jc:b * 8 + jc + 1], 0.0,
                            ADD, MAX)
            if stage >= 2:
                emit_t1(range(5))

            # ---- decode MLP rest ----
            d2_ps = pmisc.tile([16, BPC * N], F32, tag="m")
            nc.tensor.matmul(d2_ps[:], dw2, h1[:])
            nc.scalar.activation(h2[:], d2_ps[:], RELU, bias=db2)
            d3_ps = pmisc.tile([32, BPC * N], F32, tag="m")
            nc.tensor.matmul(d3_ps[:], dw3, h2[:])
            nc.scalar.activation(fT[:], d3_ps[:], RELU, bias=db3)

            if stage == 1:
                dbg = wpool.tile([C2, BPC * N], BF16, tag="dbg")
                nc.vector.tensor_scalar(dbg[:], fT[0:16, :], 0.0, None, ADD)
                nc.sync.dma_start(out_d, dbg[:])
            if stage == 2:
                dbg = wpool.tile([C2, BPC * N], BF16, tag="dbg")
                nc.vector.tensor_scalar(dbg[:], t1t[0][0:16, :], 0.0, None, ADD)
                nc.sync.dma_start(out_d, dbg[:])

            # ---- g stage + bounce ----
            if stage >= 4:
                g_ps = pg.tile([128, BPC * 256], F32, tag="g")
                g_sb = wpool.tile([128, BPC * 256], BF16, tag="gsb")
                g_sb_v = g_sb[:].rearrange("p (k b d) -> p b k d",
                                           k=16, b=BPC, d=16)
                for b in range(BPC):
                    nc.tensor.matmul(g_ps[:, b * 256:(b + 1) * 256],
                                     fT[0:32, b * N:(b + 1) * N], kw3p)
                for b in range(BPC):
                    nc.vector.tensor_copy(
                        g_sb_v[:, b],
                        g_ps[:, b * 256:(b + 1) * 256].rearrange(
                            "p (k d) -> p k d", d=16))
                nc.sync.dma_start(g_dram.rearrange("(j f) -> j f", j=128),
                                  g_sb[:])
                nc.sync.dma_start(
                    g_all[:].rearrange("p (c f) -> p c f", c=16, f=64),
                    g_dram.rearrange("(c p f) -> p c f", c=16, p=128, f=64))
            if stage >= 2:
                emit_t1(range(5, 8))

            # ---- F / bias2 (DVE after t1; tiny PE mm) ----
            if stage >= 5:
                F_f32 = wpool.tile([32, BPC], F32, tag="Ff")
                for b in range(BPC):
                    nc.vector.tensor_reduce(F_f32[:, b:b + 1],
                                            fT[0:32, b * N:(b + 1) * N],
                                            mybir.AxisListType.X, ADD)
                F_sb = wpool.tile([32, BPC], BF16, tag="F")
                nc.vector.tensor_copy(F_sb[:], F_f32[:])
                b2T_sb = wpool.tile([16, BPC], F32, tag="b2T")
                b2T_ps = pmisc.tile([16, BPC], F32, tag="m")
                nc.tensor.matmul(b2T_ps[:], kb3r, F_sb[:])
                nc.vector.tensor_copy(b2T_sb[:], b2T_ps[:])

            # ---- t2 (16 mm, 8 Act drains) + interleaved finals ----
            if stage >= 3:
                def emit_final(c2, stop):
                    for b in range(BPC):
                        nc.tensor.matmul(
                            ot_ps[:, b * N:(b + 1) * N],
                            g_all[:, c2 * 64 + b * 16:c2 * 64 + (b + 1) * 16],
                            t2ball[:, c2 * 512 + b * N:
                                   c2 * 512 + (b + 1) * N],
                            start=False, stop=True)

                for jc in range(8):
                    ps = pt2.tile([128, 1024], F32, tag="t2ps")
                    for half in range(2):
                        nc.tensor.matmul(
                            ps[:, half * 512:(half + 1) * 512],
                            W2[:, half * 128:(half + 1) * 128],
                            t1t[jc][:])
                    nc.scalar.activation(
                        t2ball[:, jc * 1024:(jc + 1) * 1024],
                        ps[:], RELU, bias=kb2t)
                    if stage >= 5 and jc >= 1:
                        emit_final(2 * (jc - 1), False)
                        emit_final(2 * (jc - 1) + 1, False)
                if stage >= 5:
                    emit_final(14, False)
                    emit_final(15, True)
                    out_sb = wpool.tile([16, BPC * N], BF16, tag="osb")
                    for b in range(BPC):
                        nc.scalar.activation(out_sb[:, b * N:(b + 1) * N],
                                             ot_ps[:, b * N:(b + 1) * N],
                                             RELU, bias=b2T_sb[:, b:b + 1])
                    nc.sync.dma_start(out_d, out_sb[:])

            if stage == 3:
                dbg = wpool.tile([C2, BPC * N], BF16, tag="dbg")
                nc.vector.tensor_scalar(dbg[:], t2ball[0:16, 0:512], 0.0,
                                        None, ADD)
                nc.sync.dma_start(out_d, dbg[:])
            if stage == 4:
                dbg = wpool.tile([C2, BPC * N], BF16, tag="dbg")
                nc.vector.tensor_scalar(dbg[:], g_all[0:16, 0:512], 0.0,
                                        None, ADD)
                nc.sync.dma_start(out_d, dbg[:])

    nc.compile()
    return nc


def _host_inputs(feature, coordinates_v, dw1, db1, dw2, db2, dw3, db3,
                 kw1, kb1, kw2, kb2, kw3, kb3):
    """Per-core input maps. Pure layout transforms, no FLOPs."""
    f32 = np.float32
    wbs = np.zeros((67, 352), f32)
    wbs[0:67, 0:32] = dw1
    wbs[0:32, 32:48] = dw2
    wbs[0:16, 48:80] = dw3
    wbs[0:3, 80:208] = np.tile(-np.asarray(kw1), (1, 16))
    kw1b = np.concatenate([np.asarray(kw1), np.asarray(kb1)[None, :]], 0)
    for jl in range(16):
        wbs[jl * 4:(jl + 1) * 4, 208 + jl * 8:208 + (jl + 1) * 8] = kw1b
    wbs[0:32, 336:352] = np.asarray(kb3).reshape(32, 16)
    wbs = wbs.astype(BNP)

    wbb = np.zeros((128, 512), f32)
    for half in range(2):
        for jl8 in range(8):
            jl = half * 8 + jl8
            wbb[jl * 8:(jl + 1) * 8, half * 128 + jl8 * 16:
                half * 128 + (jl8 + 1) * 16] = kw2
    wbb[0:32, 256:512] = np.asarray(kw3).reshape(16, 32, 16).transpose(
        1, 0, 2).reshape(32, 256)
    wbb = wbb.astype(BNP)

    wf = np.zeros((128, 4), f32)
    wf[0:32, 0] = db1
    wf[0:16, 1] = db2
    wf[0:32, 2] = db3
    wf[:, 3] = np.tile(kb2, 8)

    in_maps = []
    for c in range(NCORES):
        fe = feature[c * BPC:(c + 1) * BPC]          # [4, 64]
        co = coordinates_v[c * BPC:(c + 1) * BPC]    # [4, 128, 3]
        xT = np.empty((67, BPC * N), f32)
        c4T = np.zeros((4, BPC * N), f32)
        for b in range(BPC):
            xT[0:64, b * N:(b + 1) * N] = fe[b][:, None]
            xT[64:67, b * N:(b + 1) * N] = co[b].T
            c4T[0:3, b * N:(b + 1) * N] = co[b].T
        c4T2 = np.empty((64, BPC * 8), f32)
        v = co.reshape(BPC, 8, 16, 3)                # [b, jc, jl, x]
        for jl in range(16):
            c4T2[jl * 4:jl * 4 + 3, :] = v[:, :, jl, :].transpose(
                2, 0, 1).reshape(3, BPC * 8)
            c4T2[jl * 4 + 3, :] = 1.0
        in_maps.append({"xT": xT.astype(BNP), "c4T": c4T.astype(BNP),
                        "c4T2": c4T2.astype(BNP), "wbs": wbs, "wbb": wbb,
                        "wb_f32": wf})
    return in_maps


def kernel(**inputs):
    global _CACHED_NC
    if _CACHED_NC is None:
        _CACHED_NC = build_nc()
    nc = _CACHED_NC
    in_maps = _host_inputs(
        np.asarray(inputs["feature"]), np.asarray(inputs["coordinates_v"]),
        np.asarray(inputs["dw1"]), np.asarray(inputs["db1"]),
        np.asarray(inputs["dw2"]), np.asarray(inputs["db2"]),
        np.asarray(inputs["dw3"]), np.asarray(inputs["db3"]),
        np.asarray(inputs["kw1"]), np.asarray(inputs["kb1"]),
        np.asarray(inputs["kw2"]), np.asarray(inputs["kb2"]),
        np.asarray(inputs["kw3"]), np.asarray(inputs["kb3"]))
    res = run_bass_kernel_spmd(nc, in_maps, list(range(NCORES)))
    out = np.empty((B, N, C2), np.float32)
    for c in range(NCORES):
        r = res.results[c]["out"].astype(np.float32).reshape(C2, BPC, N)
        out[c * BPC:(c + 1) * BPC] = r.transpose(1, 2, 0)
    return out


# revision 39
# speedup vs baseline: 1.0863x; 1.0863x over previous
"""Trainium2 Bass kernel for HandDecoder-style GNN message passing.

Math (per batch element b, N=128 nodes):
  f = relu(MLP3([feature, coords]))                          # [N, C1=32]
  t1[i,j,h] = relu(a[j,h] + kb1[h] - a[i,h]),  a = coords @ kw1   # [N,N,8]
  t2[i,j,k] = relu(sum_h t1[i,j,h] kw2[h,k] + kb2[k])             # [N,N,16]
  g[j,k,d]  = sum_c f[j,c] kw3[k, c*16+d]                          # [N,16,16]
  out[i,d]  = relu(sum_{j,k} t2[i,j,k] g[j,k,d] + sum_c F[c] kb3[c*16+d])
  (F[c] = sum_j f[j,c])

v4 (v3 52us, v2 50.8us, baseline 133us):
  - rep = -a replicated, via ONE matmul with jl-tiled -kw1 lhsT (no selector
    stage); rep kept fp32 (bf16 rep errors are shared across j and sum
    coherently in the final 2048-term contraction: 5.6e-2 rel err)
  - t1: 32 DVE tensor_scalar(add,max) ops, [128,512]-tiled by jc
  - t2: 16 bf16 matmuls [128,128,512] into [128,1024] PSUM tiles (one per
    jc); 8 Act relu+bias drains of [128,1024] (fixed cost amortized);
    final matmuls interleaved into the t2 phase so the PE stays dense
  - g permute: scratch [j,k,b,d] (j-stride=16*k-stride) -> 1 clean write +
    1 affine gather read; g drains on DVE
  - bias2 accumulated into out PSUM via K=1 matmuls; single Act relu drain;
    bf16 output (host casts)
  - weight blob DMA first on sync so decode starts ~9us instead of ~14us
Data-parallel over batch: 4 batch elements per core, 8 cores.
"""

import sys
import numpy as np

for _p in ("/opt/trn_rl_repo",):
    if _p not in sys.path:
        sys.path.insert(0, _p)

import concourse.bass as bass
import concourse.bacc as bacc
import concourse.mybir as mybir
import concourse.tile as tile
from concourse.bass_utils import run_bass_kernel_spmd

import ml_dtypes

B, N = 32, 128
C0, C1, C2 = 64, 32, 16
NCORES = 8
BPC = B // NCORES          # batches per core = 4
F32 = mybir.dt.float32
BF16 = mybir.dt.bfloat16
RELU = mybir.ActivationFunctionType.Relu
COPY = mybir.ActivationFunctionType.Copy
ADD = mybir.AluOpType.add
MAX = mybir.AluOpType.max
BNP = ml_dtypes.bfloat16

_CACHED_NC = None


def build_nc(stage=5):
    import os
    stage = int(os.environ.get("KSTAGE", stage))
    nc = bacc.Bacc("TRN2", target_bir_lowering=False, debug=False,
                   num_devices=NCORES)

    # -------- DRAM I/O --------
    # wb_bf blob [128, 748] bf16:
    #   [0:256]  W2: W2[jl*8+h, half*128+jl8*16+k] = kw2[h,k] if jl==half*8+jl8
    #   [256:512] kw3p rows 0:32: kw3p[c, k*16+d] = kw3[k, c*16+d]
    #   [512:544] dw1 rows 0:67
    #   [544:560] dw2 rows 0:32
    #   [560:592] dw3 rows 0:16
    #   [592:720] kw1nrep rows 0:4: [-kw1 tiled 16x along (jl,h); row3=0]
    #   [720:848] kw1blk rows 0:64: kw1blk[jl*4+x, jl2*8+h] = kw1b[x,h]*(jl==jl2)
    #   [848:864] kb3r rows 0:32: kb3r[c, d] = kb3[c*16+d]
    wb_bf = nc.dram_tensor("wb_bf", [128, 864], BF16, kind="ExternalInput").ap()
    # wb_f32 [128, 4]: col0 db1(0:32), col1 db2(0:16), col2 db3(0:32), col3 kb2t
    wb_f32 = nc.dram_tensor("wb_f32", [128, 4], F32, kind="ExternalInput").ap()
    xT = nc.dram_tensor("xT", [67, BPC * N], BF16, kind="ExternalInput").ap()
    c4T = nc.dram_tensor("c4T", [4, BPC * N], BF16, kind="ExternalInput").ap()
    c4T2 = nc.dram_tensor("c4T2", [64, BPC * 8], BF16, kind="ExternalInput").ap()
    out_d = nc.dram_tensor("out", [C2, BPC * N], BF16, kind="ExternalOutput").ap()
    # g scratch: [j, k, b, d]-major (j-stride 1024 = 16 * k-stride 64)
    g_dram = nc.dram_tensor("gscr", [128 * BPC * 256], BF16).ap()

    with tile.TileContext(nc) as tc:
        with (
            tc.tile_pool(name="const", bufs=1) as cpool,
            tc.tile_pool(name="work", bufs=1) as wpool,
        ):
            wb = cpool.tile([128, 864], BF16, tag="wb")
            wf = cpool.tile([128, 4], F32, tag="wf")
            xT_s = cpool.tile([67, BPC * N], BF16, tag="xT")
            c4T_s = cpool.tile([4, BPC * N], BF16, tag="c4T")
            c4T2_s = cpool.tile([64, BPC * 8], BF16, tag="c4T2")
            nc.sync.dma_start(wb[:], wb_bf)
            nc.sync.dma_start(c4T_s[:], c4T)
            nc.sync.dma_start(c4T2_s[:], c4T2)
            nc.sync.dma_start(xT_s[:], xT)
            nc.scalar.dma_start(wf[:], wb_f32)

            W2 = wb[0:128, 0:256]
            kw3p = wb[0:32, 256:512]
            dw1 = wb[0:67, 512:544]
            dw2 = wb[0:32, 544:560]
            dw3 = wb[0:16, 560:592]
            kw1nrep = wb[0:4, 592:720]
            kw1blk = wb[0:64, 720:848]
            kb3r = wb[0:32, 848:864]
            db1 = wf[0:32, 0:1]
            db2 = wf[0:16, 1:2]
            db3 = wf[0:32, 2:3]
            kb2t = wf[0:128, 3:4]

            rep_sb = wpool.tile([128, BPC * N], F32, tag="repsb")
            a2bT_sb = wpool.tile([128, BPC * 8], F32, tag="a2bT")
            h1 = wpool.tile([32, BPC * N], BF16, tag="h1")
            h2 = wpool.tile([16, BPC * N], BF16, tag="h2")
            fT = wpool.tile([32, BPC * N], BF16, tag="fT")

            # ---- early PE stage under a scoped psum pool ----
            with tc.tile_pool(name="ps_misc", bufs=2,
                              space=bass.MemorySpace.PSUM) as pmisc:
                # rep[(jl,h), (b,i)] = -a[h, (b,i)]   (one matmul, K=4)
                rep_ps = pmisc.tile([128, BPC * N], F32, tag="m")
                nc.tensor.matmul(rep_ps[:], kw1nrep, c4T_s[:])
                nc.vector.tensor_copy(rep_sb[:], rep_ps[:])
                # a2bT[(jl,h), (b,jc)] = (coords@kw1 + kb1) at j=jc*16+jl
                a2bT_ps = pmisc.tile([128, BPC * 8], F32, tag="m")
                nc.tensor.matmul(a2bT_ps[:], kw1blk, c4T2_s[:])
                nc.vector.tensor_copy(a2bT_sb[:], a2bT_ps[:])
                # decode MLP (Act drains)
                d1_ps = pmisc.tile([32, BPC * N], F32, tag="m")
                nc.tensor.matmul(d1_ps[:], dw1, xT_s[:])
                nc.scalar.activation(h1[:], d1_ps[:], RELU, bias=db1)
                d2_ps = pmisc.tile([16, BPC * N], F32, tag="m")
                nc.tensor.matmul(d2_ps[:], dw2, h1[:])
                nc.scalar.activation(h2[:], d2_ps[:], RELU, bias=db2)
                d3_ps = pmisc.tile([32, BPC * N], F32, tag="m")
                nc.tensor.matmul(d3_ps[:], dw3, h2[:])
                nc.scalar.activation(fT[:], d3_ps[:], RELU, bias=db3)

            if stage == 1:
                dbg = wpool.tile([C2, BPC * N], BF16, tag="dbg")
                nc.vector.tensor_scalar(dbg[:], fT[0:16, :], 0.0, None, ADD)
                nc.sync.dma_start(out_d, dbg[:])

            # ---- t1: 8 tiles [128,(b,i)=512], 32 DVE ops ----
            t1t = [wpool.tile([128, BPC * N], BF16, name=f"t1_{jc}",
                              tag=f"t1_{jc}") for jc in range(8)]

            def emit_t1(jc):
                for b in range(BPC):
                    nc.vector.tensor_scalar(
                        t1t[jc][:, b * N:(b + 1) * N],
                        rep_sb[:, b * N:(b + 1) * N],
                        a2bT_sb[:, b * 8 + jc:b * 8 + jc + 1], 0.0, ADD, MAX)

            if stage >= 2:
                emit_t1(0)
                emit_t1(1)

            # ---- g stage + bounce (DVE drains; hides under t1/t2) ----
            g_all = wpool.tile([128, BPC * 256], BF16, tag="gall")
            if stage >= 4:
                with tc.tile_pool(name="ps_g", bufs=1,
                                  space=bass.MemorySpace.PSUM) as pg:
                    g_ps = pg.tile([128, BPC * 256], F32, tag="g")
                    g_sb = wpool.tile([128, BPC * 256], BF16, tag="gsb")
                    g_sb_v = g_sb[:].rearrange("p (k b d) -> p b k d",
                                               k=16, b=BPC, d=16)
                    for b in range(BPC):
                        nc.tensor.matmul(g_ps[:, b * 256:(b + 1) * 256],
                                         fT[0:32, b * N:(b + 1) * N], kw3p)
                    for b in range(BPC):
                        nc.vector.tensor_copy(
                            g_sb_v[:, b],
                            g_ps[:, b * 256:(b + 1) * 256].rearrange(
                                "p (k d) -> p k d", d=16))
                nc.sync.dma_start(g_dram.rearrange("(j f) -> j f", j=128),
                                  g_sb[:])
                nc.sync.dma_start(
                    g_all[:].rearrange("p (c f) -> p c f", c=16, f=64),
                    g_dram.rearrange("(c p f) -> p c f", c=16, p=128, f=64))

            if stage >= 2:
                for jc in range(2, 8):
                    emit_t1(jc)

            if stage == 2:
                dbg = wpool.tile([C2, BPC * N], BF16, tag="dbg")
                nc.vector.tensor_scalar(dbg[:], t1t[0][0:16, :], 0.0, None, ADD)
                nc.sync.dma_start(out_d, dbg[:])

            # ---- bias2 path (DVE + tiny PE) ----
            if stage >= 5:
                F_f32 = wpool.tile([32, BPC], F32, tag="Ff")
                for b in range(BPC):
                    nc.vector.tensor_reduce(F_f32[:, b:b + 1],
                                            fT[0:32, b * N:(b + 1) * N],
                                            mybir.AxisListType.X, ADD)
                F_sb = wpool.tile([32, BPC], BF16, tag="F")
                nc.vector.tensor_copy(F_sb[:], F_f32[:])
                b2T_sb = wpool.tile([16, BPC], F32, tag="b2T")

            # ---- t2 + interleaved final ----
            if stage >= 3 or stage > 30:
                t2ball = wpool.tile([128, BPC * 2048], BF16, tag="t2ball")
                with (
                    tc.tile_pool(name="ps_t2", bufs=2,
                                 space=bass.MemorySpace.PSUM) as pt2,
                    tc.tile_pool(name="ps_out", bufs=1,
                                 space=bass.MemorySpace.PSUM) as pout,
                ):
                    if stage >= 5:
                        ot_ps = pout.tile([16, BPC * N], F32, tag="ot")

                    def emit_chain(b):
                        for c2 in range(16):
                            nc.tensor.matmul(
                                ot_ps[:, b * N:(b + 1) * N],
                                g_all[:, c2 * 64 + b * 16:
                                      c2 * 64 + (b + 1) * 16],
                                t2ball[:, c2 * 512 + b * N:
                                       c2 * 512 + (b + 1) * N],
                                start=(c2 == 0), stop=(c2 == 15))

                    for jc in range(8):
                        ps = pt2.tile([128, 1024], F32, tag="t2ps")
                        for half in range(2):
                            nc.tensor.matmul(
                                ps[:, half * 512:(half + 1) * 512],
                                W2[:, half * 128:(half + 1) * 128],
                                t1t[jc][:])
                        nc.scalar.activation(
                            t2ball[:, jc * 1024:(jc + 1) * 1024],
                            ps[:], RELU, bias=kb2t)
                    if stage >= 5:
                        for b in range(BPC):
                            emit_chain(b)
                        b2T_ps = pt2.tile([16, BPC], F32, tag="b2Tps")
                        nc.tensor.matmul(b2T_ps[:], kb3r, F_sb[:])
                        nc.vector.tensor_copy(b2T_sb[:], b2T_ps[:])
                        out_sb = wpool.tile([16, BPC * N], BF16, tag="osb")
                        for b in range(BPC):
                            nc.scalar.activation(out_sb[:, b * N:(b + 1) * N],
                                                 ot_ps[:, b * N:(b + 1) * N],
                                                 RELU,
                                                 bias=b2T_sb[:, b:b + 1])
                        nc.sync.dma_start(out_d, out_sb[:])

            if stage == 3:
                dbg = wpool.tile([C2, BPC * N], BF16, tag="dbg")
                nc.vector.tensor_scalar(dbg[:], t2ball[0:16, 0:512], 0.0,
                                        None, ADD)
                nc.sync.dma_start(out_d, dbg[:])
            if stage == 31:
                dbg = wpool.tile([C2, BPC * N], BF16, tag="dbg")
                nc.vector.tensor_scalar(dbg[:], t2ball[0:16, 512:1024], 0.0,
                                        None, ADD)
                nc.sync.dma_start(out_d, dbg[:])
            if stage == 32:
                nc.sync.dma_start(out_d, t2ball[112:128, 0:512])
            if stage == 33:
                dbg = wpool.tile([C2, BPC * N], F32, tag="dbg")
                nc.vector.memset(dbg[:], 0.0)
                nc.vector.tensor_copy(dbg[0:16, 0:BPC], b2T_sb[:])
                nc.vector.tensor_copy(dbg[0:16, 8:8 + BPC],
                                      F_f32[0:16, :])
                dbg16 = wpool.tile([C2, BPC * N], BF16, tag="dbg16")
                nc.vector.tensor_copy(dbg16[:], dbg[:])
                nc.sync.dma_start(out_d, dbg16[:])
            if stage == 4:
                dbg = wpool.tile([C2, BPC * N], BF16, tag="dbg")
                nc.vector.tensor_scalar(dbg[:], g_all[0:16, 0:512], 0.0,
                                        None, ADD)
                nc.sync.dma_start(out_d, dbg[:])

    nc.compile()
    return nc


def _host_inputs(feature, coordinates_v, dw1, db1, dw2, db2, dw3, db3,
                 kw1, kb1, kw2, kb2, kw3, kb3):
    """Per-core input maps. Pure layout transforms, no FLOPs."""
    f32 = np.float32
    wb = np.zeros((128, 864), f32)
    for half in range(2):
        for jl8 in range(8):
            jl = half * 8 + jl8
            wb[jl * 8:(jl + 1) * 8, half * 128 + jl8 * 16:
               half * 128 + (jl8 + 1) * 16] = kw2
    wb[0:32, 256:512] = np.asarray(kw3).reshape(16, 32, 16).transpose(
        1, 0, 2).reshape(32, 256)
    wb[0:67, 512:544] = dw1
    wb[0:32, 544:560] = dw2
    wb[0:16, 560:592] = dw3
    wb[0:3, 592:720] = np.tile(-np.asarray(kw1), (1, 16))
    kw1b = np.concatenate([np.asarray(kw1), np.asarray(kb1)[None, :]], 0)
    for jl in range(16):
        wb[jl * 4:(jl + 1) * 4, 720 + jl * 8:720 + (jl + 1) * 8] = kw1b
    wb[0:32, 848:864] = np.asarray(kb3).reshape(32, 16)
    wb = wb.astype(BNP)

    wf = np.zeros((128, 4), f32)
    wf[0:32, 0] = db1
    wf[0:16, 1] = db2
    wf[0:32, 2] = db3
    wf[:, 3] = np.tile(kb2, 8)

    in_maps = []
    for c in range(NCORES):
        fe = feature[c * BPC:(c + 1) * BPC]          # [4, 64]
        co = coordinates_v[c * BPC:(c + 1) * BPC]    # [4, 128, 3]
        xT = np.empty((67, BPC * N), f32)
        c4T = np.zeros((4, BPC * N), f32)
        for b in range(BPC):
            xT[0:64, b * N:(b + 1) * N] = fe[b][:, None]
            xT[64:67, b * N:(b + 1) * N] = co[b].T
            c4T[0:3, b * N:(b + 1) * N] = co[b].T
        c4T2 = np.empty((64, BPC * 8), f32)
        v = co.reshape(BPC, 8, 16, 3)                # [b, jc, jl, x]
        for jl in range(16):
            c4T2[jl * 4:jl * 4 + 3, :] = v[:, :, jl, :].transpose(
                2, 0, 1).reshape(3, BPC * 8)
            c4T2[jl * 4 + 3, :] = 1.0
        in_maps.append({"xT": xT.astype(BNP), "c4T": c4T.astype(BNP),
                        "c4T2": c4T2.astype(BNP), "wb_bf": wb, "wb_f32": wf})
    return in_maps


def kernel(**inputs):
    global _CACHED_NC
    if _CACHED_NC is None:
        _CACHED_NC = build_nc()
    nc = _CACHED_NC
    in_maps = _host_inputs(
        np.asarray(inputs["feature"]), np.asarray(inputs["coordinates_v"]),
        np.asarray(inputs["dw1"]), np.asarray(inputs["db1"]),
        np.asarray(inputs["dw2"]), np.asarray(inputs["db2"]),
        np.asarray(inputs["dw3"]), np.asarray(inputs["db3"]),
        np.asarray(inputs["kw1"]), np.asarray(inputs["kb1"]),
        np.asarray(inputs["kw2"]), np.asarray(inputs["kb2"]),
        np.asarray(inputs["kw3"]), np.asarray(inputs["kb3"]))
    res = run_bass_kernel_spmd(nc, in_maps, list(range(NCORES)))
    out = np.empty((B, N, C2), np.float32)
    for c in range(NCORES):
        r = res.results[c]["out"].astype(np.float32).reshape(C2, BPC, N)
        out[c * BPC:(c + 1) * BPC] = r.transpose(1, 2, 0)
    return out
